# revision 25
# baseline (speedup 1.0000x reference)
"""DGCNN classification forward pass on 8 Trainium2 NeuronCores.

Data-parallel over batch: one sample per core (B=8). Hardcoded for:
  x: [8, 3, 4096] f32, K=20 neighbors, EdgeConv channels 3->64->64->128->256,
  emb=1024, head 2048->512->256->40.

Exact algebraic restructuring (BN scale > 0, leaky-relu monotone):
  max_k lrelu(BN(W [x_nbr - x_i; x_i])) =
  lrelu( max_k (Wn' x_nbr)  +  (Wc' - Wn') x_i + b' )
so each EdgeConv is two small dense matmuls plus a gather+max over the kNN
set, and only the *set* of top-20 indices matters (max is order-invariant).

kNN per 128-row block: distance rows s[i,j] = 2<x_i,x_j> - |x_j|^2 from PE
(per-row constant |x_i|^2 dropped -- row-wise ordering unaffected), then a
segmented DVE top-8 (16 segments x 256) whose 128 candidates contain the
top-20 with probability 1 - ~2e-5 per row; merge via max8/match_replace for
the 20th value, then extract the 20 positions with a masked max8 chain.
"""

import math
import numpy as np

import concourse.bass as bass
import concourse.mybir as mybir
from concourse import library_config
from concourse.tile import TileContext
from concourse.bass_utils import run_bass_kernel_spmd

F32 = mybir.dt.float32
I16 = mybir.dt.int16
U16 = mybir.dt.uint16
F32R = mybir.dt.float32r
ALU = mybir.AluOpType
ACTF = mybir.ActivationFunctionType
AX = mybir.AxisListType

N = 4096
KNN = 20
NBLK = N // 128          # 32 point blocks
NCHUNK = N // 512        # 8 psum chunks per row-block
SEG = 256                # top-k segment width
NSEG = N // SEG          # 16 segments -> 128 candidates/row
NCAND = NSEG * 8
GRP = 4                  # blocks per fold/gather group
HLF = 2                  # gather halves per block
SLOPE = 0.2
EPS = 1e-5
BN_INV = 1.0 / math.sqrt(1.0 + EPS)
NEG = -1.0e9

# (C_in, C_out, aug): aug -> x stored with a ones row at row C_in; the dist
# matmul is a single augmented matmul.  Layer 4 (C_in=128) has no room for
# the ones row and uses a separate K=1 accumulate instead.
LAYERS = [(3, 64, True), (64, 64, True), (64, 128, True), (128, 256, False)]


def _leaky(nc, out_ap, in_ap, accum_out=None):
    # leaky_relu(x) = max(x, 0.2*x)   (in_ap must be SBUF)
    nc.vector.scalar_tensor_tensor(
        out=out_ap, in0=in_ap, scalar=SLOPE, in1=in_ap,
        op0=ALU.mult, op1=ALU.max, accum_out=accum_out)


def _leaky_psum(nc, out_ap, ps_ap, tmp_ap, accum_out=None):
    # PSUM-input variant: only one non-scalar PSUM input allowed per DVE op
    nc.vector.tensor_scalar(out=tmp_ap, in0=ps_ap, scalar1=SLOPE,
                            scalar2=None, op0=ALU.mult)
    nc.vector.scalar_tensor_tensor(
        out=out_ap, in0=ps_ap, scalar=1.0, in1=tmp_ap,
        op0=ALU.mult, op1=ALU.max, accum_out=accum_out)


def split_waits(nc, maxw=1):
    """walrus CoreV3 codegen rejects >1 sync wait per instruction; the Tile
    tail drain carries one wait per live proc.  Hoist the excess onto extra
    Drain instructions inserted immediately before."""
    nsplit = 0
    for _, bb in list(nc.bb_map.items()):
        bbi = bb.bb if hasattr(bb, "bb") else bb
        insts = bbi.instructions
        i = 0
        while i < len(insts):
            inst = insts[i]
            si = inst.sync_info
            if si is not None and si.on_wait is not None and len(si.on_wait) > maxw:
                waits = list(si.on_wait)
                chunks = [waits[j:j + maxw] for j in range(0, len(waits), maxw)]
                new_insts = []
                for ch in chunks[:-1]:
                    d = mybir.InstDrain(
                        name=nc.get_next_instruction_name(), ins=[], outs=[],
                        bass_is_fusable=False)
                    d.engine = inst.engine
                    d.sync_info = mybir.SyncInfo(on_wait=ch, on_update=[])
                    nc.inst_map[d.name] = d
                    new_insts.append(d)
                si.on_wait = chunks[-1]
                inst.sync_info = si
                insts[i:i] = new_insts
                i += len(new_insts)
                nsplit += 1
            i += 1
    return nsplit


def build_nc():
    nc = bass.Bass("TRN2", target_bir_lowering=False, debug=False, num_devices=8)

    dp = lambda name, shape, dt=F32: nc.declare_dram_parameter(
        name, list(shape), dt, isOutput=False)

    x_in = dp("x", [3, N])
    onespk = dp("onespk", [128, 513])     # [:,0] ones col; [0,1:513] ones row
    offs_in = dp("offs", [128, NCAND], U16)

    wpk = []
    for li, (ci, co, aug) in enumerate(LAYERS):
        # engine SBUF APs must start at partition 0/32/64/96, so layer 1's
        # augmented ones-row sits at partition 32 with zero rows 3..31
        ar = 32 if (aug and li == 0) else ci
        cia = ar + 1 if aug else ci
        # packed per-layer weights: [:ci,0:co]=WnT, [:cia,co:2co]=WzT(+bias row
        # for aug), [0,2co:3co]=z bias row (non-aug only)
        wpk.append(dp(f"wpk{li}", [cia, 3 * co]))
    w5T = dp("w5T", [512, 1024])
    b5 = dp("b5", [1, 1024])
    wl1T = dp("wl1T", [2048, 512])
    bl1 = dp("bl1", [1, 512])
    wl2T = dp("wl2T", [512, 256])
    bl2 = dp("bl2", [1, 256])
    wl3T = dp("wl3T", [256, 40])
    bl3 = dp("bl3", [1, 40])

    out_t = nc.declare_dram_parameter("out", [40, 1], F32, isOutput=True)

    cat_dims = [64, 64, 128, 128, 128]
    cats = [nc.dram_tensor(f"cat{i}", [cat_dims[i], N], F32) for i in range(5)]

    with TileContext(nc) as tc:
        nc.gpsimd.load_library(library_config.ap_gather)
        with tc.tile_pool(name="const", bufs=1) as constp, \
             tc.tile_pool(name="persist", bufs=1) as persist:

            onest = constp.tile([128, 513], F32, tag="onespk")
            nc.sync.dma_start(out=onest, in_=onespk[:, :])
            ones_c = onest[:, 0:1]          # [128,1] ones
            ones_s = onest[0:1, 1:513]      # [1,512] ones
            offs_t = constp.tile([128, NCAND], U16, tag="offs")
            nc.sync.dma_start(out=offs_t, in_=offs_in[:, :])

            pool2 = persist.tile([128, 16], F32, tag="pool2")
            amp = pool2[:, 0:8]
            aap = pool2[:, 8:16]

            # ---------------- phase A: the 4 EdgeConv layers ----------------
            with tc.tile_pool(name="xs", bufs=3) as xsp, \
                 tc.tile_pool(name="yp", bufs=1) as yp, \
                 tc.tile_pool(name="lw", bufs=1) as lwp, \
                 tc.tile_pool(name="ra", bufs=1) as rap, \
                 tc.tile_pool(name="work", bufs=2) as wp, \
                 tc.tile_pool(name="sbp", bufs=3) as sbp, \
                 tc.tile_pool(name="sqp", bufs=1) as sqp, \
                 tc.tile_pool(name="cand", bufs=2) as cp, \
                 tc.tile_pool(name="pd", bufs=5, space="PSUM") as pdist, \
                 tc.tile_pool(name="px", bufs=1, space="PSUM") as pxx, \
                 tc.tile_pool(name="pz", bufs=2, space="PSUM") as pz:

                xs_cur = xsp.tile([128, N], F32, tag="xs")
                nc.gpsimd.memset(xs_cur[0:33, :], 0.0)
                nc.sync.dma_start(out=xs_cur[0:3, :], in_=x_in[:, :])
                nc.gpsimd.memset(xs_cur[32:33, :], 1.0)

                for li, (ci, co, aug) in enumerate(LAYERS):
                    ar = 32 if (aug and li == 0) else ci
                    cia = ar + 1 if aug else ci
                    # ---- per-layer prep ----
                    wpkt = lwp.tile([cia, 3 * co], F32, tag="wpk")
                    nc.sync.dma_start(out=wpkt, in_=wpk[li][:, :])
                    wn_t = wpkt[0:ci, 0:co]
                    wz_t = wpkt[0:cia, co:2 * co]
                    zb_t = wpkt[0:1, 2 * co:3 * co]

                    # y = Wn' x  in [co, N] layout (gather source)
                    ya = yp.tile([128, N], F32, tag="ya")
                    if co > 128:
                        yb = yp.tile([128, N], F32, tag="yb")
                    else:
                        yb = None
                    for mb in range((co + 127) // 128):
                        mco = min(128, co - mb * 128)
                        ydst = ya if mb == 0 else yb
                        for c in range(NCHUNK):
                            ps = pdist.tile([128, 512], F32, tag="pd")
                            nc.tensor.matmul(
                                ps[0:mco, :], wn_t[:, mb * 128:mb * 128 + mco],
                                xs_cur[0:ci, c * 512:(c + 1) * 512],
                                start=True, stop=True)
                            nc.scalar.copy(out=ydst[0:mco, c * 512:(c + 1) * 512],
                                           in_=ps[0:mco, :])

                    # rhs rows 0..ci-1 = 2*x ; row ci (aug) or nxx = -|x_j|^2
                    ra = rap.tile([128, N], F32, tag="ra")
                    if not aug:
                        nxx = rap.tile([1, N], F32, tag="nxx")
                    else:
                        nxx = None
                    if ar != ci:
                        nc.gpsimd.memset(ra[0:33, :], 0.0)
                    nc.scalar.activation(out=ra[0:ci, :], in_=xs_cur[0:ci, :],
                                         func=ACTF.Copy, scale=2.0)
                    for c in range(NCHUNK):
                        sq = sqp.tile([128, 512], F32, tag="sq")
                        nc.scalar.square(out=sq[0:ci, :],
                                         in_=xs_cur[0:ci, c * 512:(c + 1) * 512])
                        px = pxx.tile([1, 512], F32, tag="px")
                        nc.tensor.matmul(px, ones_c[0:ci, :], sq[0:ci, :],
                                         start=True, stop=True)
                        xxdst = ra[ar:ar + 1, c * 512:(c + 1) * 512] if aug \
                            else nxx[:, c * 512:(c + 1) * 512]
                        nc.scalar.activation(out=xxdst, in_=px,
                                             func=ACTF.Copy, scale=-1.0)

                    # output storage (+ ones row if the next layer is aug)
                    if co <= 128:
                        xs_nxt = xsp.tile([128, N], F32, tag="xs")
                        outs = [(xs_nxt, 0, co)]
                    else:
                        xs_na = xsp.tile([128, N], F32, tag="xs")
                        xs_nb = xsp.tile([128, N], F32, tag="xs")
                        outs = [(xs_na, 0, 128), (xs_nb, 128, 128)]
                    nxt_aug = (li + 1 < len(LAYERS)) and LAYERS[li + 1][2]
                    if nxt_aug and co < 128:
                        nc.gpsimd.memset(outs[0][0][co:co + 1, :], 1.0)

                    # ---- per-group-of-8-blocks pipeline ----
                    for g in range(NBLK // GRP):
                        idxt = wp.tile([128, GRP * 192], I16, tag="idx")
                        fgrp = idxt[:, 0:GRP * 32]
                        wgrp = idxt[:, GRP * 32:GRP * 192]
                        for bl in range(GRP):
                            b = g * GRP + bl
                            sblk = sbp.tile([128, N], F32, tag="sblk")
                            for c in range(NCHUNK):
                                ps = pdist.tile([128, 512], F32, tag="pd")
                                if aug:
                                    nc.tensor.matmul(
                                        ps, xs_cur[0:cia, b * 128:(b + 1) * 128],
                                        ra[0:cia, c * 512:(c + 1) * 512],
                                        start=True, stop=True)
                                else:
                                    nc.tensor.matmul(
                                        ps, xs_cur[0:ci, b * 128:(b + 1) * 128],
                                        ra[0:ci, c * 512:(c + 1) * 512],
                                        start=True, stop=False)
                                    nc.tensor.matmul(
                                        ps, ones_s[:, 0:128],
                                        nxx[:, c * 512:(c + 1) * 512],
                                        start=False, stop=True)
                                nc.scalar.copy(out=sblk[:, c * 512:(c + 1) * 512],
                                               in_=ps)

                            # --- top-20 of each row ---
                            scr = cp.tile([128, 688], F32, tag="scr")
                            segv = scr[:, 0:128]
                            sv2 = scr[:, 128:256]
                            sv3 = scr[:, 256:384]
                            posf = scr[:, 384:512]
                            mneg = scr[:, 560:688]
                            m1 = scr[:, 512:520]
                            m2 = scr[:, 520:528]
                            m3 = scr[:, 528:536]
                            fpos = scr[:, 536:560]
                            msk1 = sv3
                            msk2 = segv
                            msk3 = sv2
                            scri = cp.tile([128, 256], U16, tag="scri")
                            segi = scri[:, 0:128]
                            cpos = scri[:, 128:256]

                            for s in range(NSEG):
                                nc.vector.max(out=segv[:, 8 * s:8 * s + 8],
                                              in_=sblk[:, SEG * s:SEG * (s + 1)])
                            for s in range(NSEG):
                                nc.vector.max_index(
                                    out=segi[:, 8 * s:8 * s + 8],
                                    in_max=segv[:, 8 * s:8 * s + 8],
                                    in_values=sblk[:, SEG * s:SEG * (s + 1)])
                            nc.vector.tensor_tensor(out=cpos, in0=segi,
                                                    in1=offs_t, op=ALU.add)
                            nc.vector.tensor_copy(out=posf, in_=cpos)

                            nc.vector.max(out=m1, in_=segv)
                            nc.vector.match_replace(out=sv2, in_to_replace=m1,
                                                    in_values=segv, imm_value=NEG)
                            nc.vector.max(out=m2, in_=sv2)
                            nc.vector.match_replace(out=sv3, in_to_replace=m2,
                                                    in_values=sv2, imm_value=NEG)
                            nc.vector.max(out=m3, in_=sv3)
                            # mneg = (segv < t20) * NEG   (t20 = m3[:,3])
                            nc.vector.tensor_scalar(
                                out=mneg, in0=segv, scalar1=m3[:, 3:4],
                                scalar2=NEG, op0=ALU.is_lt, op1=ALU.mult)
                            nc.vector.tensor_tensor(out=msk1, in0=posf,
                                                    in1=mneg, op=ALU.add)
                            nc.vector.max(out=fpos[:, 0:8], in_=msk1)
                            nc.vector.match_replace(out=msk2,
                                                    in_to_replace=fpos[:, 0:8],
                                                    in_values=msk1, imm_value=NEG)
                            nc.vector.max(out=fpos[:, 8:16], in_=msk2)
                            nc.vector.match_replace(out=msk3,
                                                    in_to_replace=fpos[:, 8:16],
                                                    in_values=msk2, imm_value=NEG)
                            nc.vector.max(out=fpos[:, 16:24], in_=msk3)
                            nc.vector.tensor_copy(
                                out=fgrp[:, 32 * bl:32 * bl + 20],
                                in_=fpos[:, 0:20])

                        # fold [128pts, 20] -> wrapped [16, GRP*160], replicate
                        for f in range(8):
                            src = fgrp[16 * f:16 * f + 16, :].rearrange(
                                "p (b k) -> p b k", b=GRP)[:, :, 0:20]
                            dst = wgrp[0:16, :].rearrange(
                                "p (b c) -> p b c", b=GRP)[:, :, 20 * f:20 * f + 20]
                            nc.sync.dma_start(out=dst, in_=src)
                        for gg in range(1, 8):
                            nc.sync.dma_start(
                                out=wgrp[16 * gg:16 * gg + 16, :],
                                in_=wgrp[0:16, :])

                        # gather + slot max + z + leaky, per block (2 halves)
                        for bl in range(GRP):
                            b = g * GRP + bl
                            pzt = pz.tile([128, 128], F32, tag="pz")
                            for mb, (xs_out, c0, mco) in enumerate(outs):
                                ch = ((mco + 15) // 16) * 16
                                ysrc = ya if mb == 0 else yb
                                sm = wp.tile([128, 256], F32, tag="sm")
                                red = sm[:, 0:128]
                                hh = sm[:, 128:256]
                                gt = wp.tile([128, KNN * 128], F32, tag="gt")
                                idxs = wgrp[:, 160 * bl:160 * (bl + 1)]
                                nc.gpsimd.ap_gather(
                                    gt[0:ch, :], ysrc[0:ch, :],
                                    idxs[0:ch, :], channels=ch,
                                    num_elems=N, d=1, num_idxs=KNN * 128)
                                gv = gt[0:mco, :].rearrange(
                                    "c (f k p) -> c f p k", f=8, k=KNN, p=16)
                                rv = red[0:mco, :].rearrange(
                                    "c (f p) -> c f p", f=8, p=16)
                                nc.vector.tensor_reduce(
                                    rv, gv, axis=AX.X, op=ALU.max)
                                # z
                                if aug:
                                    nc.tensor.matmul(
                                        pzt[0:mco, :], wz_t[:, c0:c0 + mco],
                                        xs_cur[0:cia, b * 128:(b + 1) * 128],
                                        start=True, stop=True)
                                else:
                                    nc.tensor.matmul(
                                        pzt[0:mco, :], wz_t[:, c0:c0 + mco],
                                        xs_cur[0:ci, b * 128:(b + 1) * 128],
                                        start=True, stop=False)
                                    nc.tensor.matmul(
                                        pzt[0:mco, :], zb_t[:, c0:c0 + mco],
                                        ones_s[:, 0:128],
                                        start=False, stop=True)
                                nc.vector.tensor_tensor(
                                    out=hh[0:mco, :], in0=red[0:mco, :],
                                    in1=pzt[0:mco, :], op=ALU.add)
                                _leaky(nc, xs_out[0:mco, b * 128:(b + 1) * 128],
                                       hh[0:mco, :])

                    # spill layer output for the mlp5 stage
                    if li < 3:
                        nc.sync.dma_start(out=cats[li][:, :],
                                          in_=outs[0][0][0:co, :])
                    else:
                        nc.sync.dma_start(out=cats[3][:, :], in_=outs[0][0][0:128, :])
                        nc.sync.dma_start(out=cats[4][:, :], in_=outs[1][0][0:128, :])

                    xs_cur = outs[0][0]

            # ---------------- phase B: mlp5 + pooling ----------------
            with tc.tile_pool(name="w5", bufs=1) as w5p, \
                 tc.tile_pool(name="cstream", bufs=3) as csp, \
                 tc.tile_pool(name="h5", bufs=2) as h5p, \
                 tc.tile_pool(name="acc5", bufs=1) as a5p, \
                 tc.tile_pool(name="p5", bufs=4, space="PSUM") as p5p:
                w5s = []
                k0 = 0
                for s, kd in enumerate(cat_dims):
                    t = w5p.tile([kd, 1024], F32, tag=f"w5_{s}")
                    nc.sync.dma_start(out=t, in_=w5T[k0:k0 + kd, :])
                    w5s.append(t)
                    k0 += kd
                b5t = w5p.tile([1, 1024], F32, tag="b5")
                nc.sync.dma_start(out=b5t, in_=b5[:, :])

                accs = a5p.tile([128, 128], F32, tag="accs")
                ampp = accs[:, 0:64]
                aapp = accs[:, 64:128]
                for c in range(NCHUNK):
                    catts = []
                    for s, kd in enumerate(cat_dims):
                        t = csp.tile([128, 512], F32, tag=f"cat_{s}")
                        nc.sync.dma_start(out=t[0:kd, :],
                                          in_=cats[s][:, c * 512:(c + 1) * 512])
                        catts.append(t)
                    for m in range(8):
                        ps = p5p.tile([128, 512], F32, tag="p5")
                        for s, kd in enumerate(cat_dims):
                            nc.tensor.matmul(
                                ps, w5s[s][:, m * 128:(m + 1) * 128],
                                catts[s][0:kd, :], start=(s == 0), stop=False)
                        nc.tensor.matmul(ps, b5t[:, m * 128:(m + 1) * 128],
                                         ones_s, start=False, stop=True)
                        # leaky(x) = 0.6x + 0.4|x| ; |x| on ACT frees the DVE
                        h5 = h5p.tile([128, 512], F32, tag="h5")
                        l5 = h5p.tile([128, 512], F32, tag="l5")
                        nc.scalar.activation(out=l5, in_=ps, func=ACTF.Abs,
                                             scale=0.4)
                        nc.vector.scalar_tensor_tensor(
                            out=h5, in0=ps, scalar=0.6, in1=l5,
                            op0=ALU.mult, op1=ALU.add,
                            accum_out=aapp[:, m * 8 + c:m * 8 + c + 1])
                        nc.vector.tensor_reduce(
                            ampp[:, m * 8 + c:m * 8 + c + 1], h5,
                            axis=AX.X, op=ALU.max)
                for m in range(8):
                    nc.vector.tensor_reduce(
                        amp[:, m:m + 1], ampp[:, m * 8:m * 8 + 8],
                        axis=AX.X, op=ALU.max)
                    nc.vector.tensor_reduce(
                        aap[:, m:m + 1], aapp[:, m * 8:m * 8 + 8],
                        axis=AX.X, op=ALU.add)

            # ---------------- phase C: head ----------------
            with tc.tile_pool(name="hw", bufs=1) as hwp, \
                 tc.tile_pool(name="hv", bufs=1) as hvp, \
                 tc.tile_pool(name="ph", bufs=4, space="PSUM") as php:
                wl1 = hwp.tile([128, 16 * 512], F32, tag="wl1")
                nc.sync.dma_start(
                    out=wl1.rearrange("p (s f) -> p s f", s=16),
                    in_=wl1T[:, :].rearrange("(s p) f -> p s f", p=128))
                wl2 = hwp.tile([128, 4 * 256], F32, tag="wl2")
                nc.sync.dma_start(
                    out=wl2.rearrange("p (s f) -> p s f", s=4),
                    in_=wl2T[:, :].rearrange("(s p) f -> p s f", p=128))
                wl3 = hwp.tile([128, 2 * 40], F32, tag="wl3")
                nc.sync.dma_start(
                    out=wl3.rearrange("p (s f) -> p s f", s=2),
                    in_=wl3T[:, :].rearrange("(s p) f -> p s f", p=128))
                blt = hwp.tile([1, 512 + 256 + 40], F32, tag="blt")
                nc.sync.dma_start(out=blt[:, 0:512], in_=bl1[:, :])
                nc.sync.dma_start(out=blt[:, 512:768], in_=bl2[:, :])
                nc.sync.dma_start(out=blt[:, 768:808], in_=bl3[:, :])

                hv = hvp.tile([128, 8], F32, tag="hv")
                h1 = hv[:, 0:4]
                h2 = hv[:, 4:6]
                for m in range(4):
                    ps = php.tile([128, 1], F32, tag="ph")
                    for s in range(16):
                        vec = amp[:, s:s + 1] if s < 8 else aap[:, s - 8:s - 7]
                        nc.tensor.matmul(
                            ps, wl1[:, s * 512 + m * 128:s * 512 + (m + 1) * 128],
                            vec, start=(s == 0), stop=False)
                    nc.tensor.matmul(ps, blt[:, m * 128:(m + 1) * 128],
                                     ones_s[:, 0:1], start=False, stop=True)
                    _leaky_psum(nc, h1[:, m:m + 1], ps, hv[:, 6:7])
                for m in range(2):
                    ps = php.tile([128, 1], F32, tag="ph")
                    for s in range(4):
                        nc.tensor.matmul(
                            ps, wl2[:, s * 256 + m * 128:s * 256 + (m + 1) * 128],
                            h1[:, s:s + 1], start=(s == 0), stop=False)
                    nc.tensor.matmul(ps, blt[:, 512 + m * 128:512 + (m + 1) * 128],
                                     ones_s[:, 0:1], start=False, stop=True)
                    _leaky_psum(nc, h2[:, m:m + 1], ps, hv[:, 6:7])
                pso = php.tile([40, 1], F32, tag="pho")
                for s in range(2):
                    nc.tensor.matmul(pso, wl3[:, s * 40:(s + 1) * 40],
                                     h2[:, s:s + 1], start=(s == 0), stop=False)
                nc.tensor.matmul(pso, blt[:, 768:808], ones_s[:, 0:1],
                                 start=False, stop=True)
                ot = hvp.tile([40, 1], F32, tag="ot")
                nc.scalar.copy(out=ot, in_=pso)
                nc.sync.dma_start(out=out_t[:, :], in_=ot)

    from concourse.library_overlay import lower_extended_insts
    lower_extended_insts(nc)
    split_waits(nc, 1)
    return nc


def fold_params(params):
    """Host-side: fold BN scale into weights; build all DRAM inputs."""
    p = {k: np.asarray(v, dtype=np.float32) for k, v in params.items()}
    ins = {}
    ws = [("w1", "g1", "b1"), ("w2", "g2", "b2"),
          ("w3", "g3", "b3"), ("w4", "g4", "b4")]
    for li, ((wk, gk, bk), (ci, co, aug)) in enumerate(zip(ws, LAYERS)):
        W = p[wk]                       # [co, 2ci]
        g = p[gk] * BN_INV
        b = p[bk]
        Wn = W[:, :ci] * g[:, None]
        Wc = W[:, ci:] * g[:, None]
        ar = 32 if (aug and li == 0) else ci
        cia = ar + 1 if aug else ci
        pk = np.zeros((cia, 3 * co), dtype=np.float32)
        pk[0:ci, 0:co] = Wn.T
        Z = (Wc - Wn).T
        if aug:
            pk[0:ci, co:2 * co] = Z
            pk[ar, co:2 * co] = b
        else:
            pk[0:ci, co:2 * co] = Z
            pk[0, 2 * co:3 * co] = b
        ins[f"wpk{li}"] = pk
    g5 = p["g5"] * BN_INV
    ins["w5T"] = np.ascontiguousarray((p["w5"] * g5[:, None]).T)
    ins["b5"] = p["b5"][None, :].copy()
    g1 = p["lg1"] * BN_INV
    Wl1 = (p["lw1"] * g1[:, None]).copy()
    Wl1[:, 1024:] /= float(N)
    ins["wl1T"] = np.ascontiguousarray(Wl1.T)
    ins["bl1"] = p["lb1"][None, :].copy()
    g2 = p["lg2"] * BN_INV
    ins["wl2T"] = np.ascontiguousarray((p["lw2"] * g2[:, None]).T)
    ins["bl2"] = (g2 * p["lbias2"] + p["lb2"])[None, :].copy()
    ins["wl3T"] = np.ascontiguousarray(p["lw3"].T)
    ins["bl3"] = p["lbias3"][None, :].copy()

    onespk = np.zeros((128, 513), dtype=np.float32)
    onespk[:, 0] = 1.0
    onespk[0, 1:513] = 1.0
    ins["onespk"] = onespk
    off = (np.arange(NCAND) // 8 * SEG).astype(np.uint16)
    ins["offs"] = np.broadcast_to(off, (128, NCAND)).copy()
    return ins


_NC_CACHE = {}


def kernel(x, params):
    x = np.asarray(x, dtype=np.float32)
    assert x.shape == (8, 3, N)
    if "nc" not in _NC_CACHE:
        _NC_CACHE["nc"] = build_nc()
    nc = _NC_CACHE["nc"]
    base = fold_params(params)
    in_maps = []
    for i in range(8):
        m = dict(base)
        m["x"] = np.ascontiguousarray(x[i])
        in_maps.append(m)
    import os
    trace = bool(int(os.environ.get("DGCNN_TRACE", "0")))
    res = run_bass_kernel_spmd(nc, in_maps, list(range(8)), trace=trace)
    _NC_CACHE["last_results"] = res
    out = np.stack([res.results[i]["out"][:, 0] for i in range(8)], axis=0)
    return out
